# revision 33
# baseline (speedup 1.0000x reference)
"""Trainium2 Bass kernel for an attention-GRU cell (Bahdanau attention + GRU update).

Computation (per batch row b):
    x   = inputs @ Wi + bi
    xg  = x @ kernel + bias                       (split into x_z, x_r, x_h)
    q   = h_tm1 @ Ua + ba_u
    S   = tanh(context @ Wa + ba_w + q)           [t, U]
    sc  = S @ Va + ba_v                           [t]
    attn = softmax(sc)                            (scores bounded by ||Va||_1 -> no max-sub)
    cv  = sum_t attn * context                    [U]
    cg  = cv @ attention_kernel                   (c_z, c_r, c_h)
    z   = sigmoid(x_z + h@Rz + c_z) ; r = sigmoid(x_r + h@Rr + c_r)
    hb  = tanh(x_h + (r*h)@Rh + c_h)
    h   = z*h_tm1 + (1-z)*hb ; out = h @ Wo + bo

Sharding: batch (64) split across 8 cores, 8 batches/core, weights replicated.
Each core is fully independent (no collectives).

Key layout trick (v2): the host packs fp8e4(ctx[2w, u]) | fp8e4(ctx[2w+1, u])<<8
into a uint16 tensor [W=1024, U=512] per batch. The device runs the DMA XBAR
transpose (16x128 u16 tiles) per 128-u chunk, landing natT16 [128, KU, 1024]
u16 in SBUF whose fp8 bitcast is exactly ctx^T: natT8[p, c, t] = fp8 ctx[t,
128c+p] in natural t order. This removes all PE transposes and all ACT
PSUM->SBUF unpack copies of the baseline, and cuts ctx HBM traffic to 8.4MB
per core (fp8 instead of f32).

Per-batch pipeline:
  - scores: Wa (fp8, x16 host-scaled) stationary, natT8 moving, DoubleRow
    -> S^T chunks [128, 1024] f32 in PSUM; tanh on ACT with scale=1/16 and
    per-partition bias (q + ba_w)^T -> th16 [128, 4, 1024] f16
  - Va dot via matmul (lhsT = Va chunk [128,1]) -> scores [1, t] in PSUM
  - exp on ACT with fused accum_out -> softmax normalizer (no max-subtract:
    |score| <= ||Va||_1 ~ 8)
  - attn replicated across partitions (gpsimd partition_broadcast); ctx_vec
    on DVE via scalar_tensor_tensor with accum_out over natT8
  - gate math per 4-batch group on partitions 0..3; group-post emission is
    delayed behind the next batch's work to avoid PE head-of-line blocking
"""

import sys

if "/opt/trn_rl_repo" not in sys.path:
    sys.path.insert(0, "/opt/trn_rl_repo")

import numpy as np

import concourse.bass as bass
import concourse.mybir as mybir
import concourse.tile as tile
from concourse import bacc

F32 = mybir.dt.float32
F16 = mybir.dt.float16
U16 = mybir.dt.uint16
F8 = mybir.dt.float8e4
AF = mybir.ActivationFunctionType
OP = mybir.AluOpType

B = 64          # total batch
T = 2048        # context length
W = T // 2      # packed u16 rows
U = 512         # units
EMB = 256
NCORES = 8
BPC = B // NCORES   # batches per core
KU = U // 128       # 4 k-chunks over units


def _build_program():
    nc = bacc.Bacc("TRN2", target_bir_lowering=False, debug=False, num_devices=NCORES)

    # ---- DRAM I/O ----
    ctxp_d = nc.dram_tensor("ctxp", [BPC, W, U], U16, kind="ExternalInput").ap()
    ctxn_d = nc.dram_tensor("ctx8n", [BPC, T, U], F8, kind="ExternalInput").ap()
    inp_d = nc.dram_tensor("inp16d", [BPC, EMB], F16, kind="ExternalInput").ap()
    h0_d = nc.dram_tensor("h0", [BPC, U], F32, kind="ExternalInput").ap()
    h016_d = nc.dram_tensor("h016d", [BPC, U], F16, kind="ExternalInput").ap()

    wa8_d = nc.dram_tensor("wa8dr", [128, 2, 2, KU, 128], F8,
                           kind="ExternalInput").ap()
    wadc_d = nc.dram_tensor("wa8dc", [128, KU, KU, 128], F8,
                           kind="ExternalInput").ap()
    va8_d = nc.dram_tensor("va8dr", [128, 2, 2, 16], F8,
                           kind="ExternalInput").ap()
    ua_d = nc.dram_tensor("ua16", [U, U], F16, kind="ExternalInput").ap()
    wi_d = nc.dram_tensor("wi16", [EMB, U], F16, kind="ExternalInput").ap()
    kern_d = nc.dram_tensor("kern16", [U, 3 * U], F16, kind="ExternalInput").ap()
    rec_d = nc.dram_tensor("rec16", [U, 3 * U], F16, kind="ExternalInput").ap()
    attk_d = nc.dram_tensor("attk16", [U, 3 * U], F16, kind="ExternalInput").ap()
    wo_d = nc.dram_tensor("wo16", [U, U], F16, kind="ExternalInput").ap()
    id_d = nc.dram_tensor("ident16", [128, 128], F16, kind="ExternalInput").ap()

    bi_d = nc.dram_tensor("bi", [U], F32, kind="ExternalInput").ap()
    bg_d = nc.dram_tensor("biasg", [3 * U], F32, kind="ExternalInput").ap()
    bau_d = nc.dram_tensor("ba_u", [U], F32, kind="ExternalInput").ap()
    bawt_d = nc.dram_tensor("ba_wt8", [128, KU, BPC], F32, kind="ExternalInput").ap()
    bav_d = nc.dram_tensor("ba_v1", [1, 1], F32, kind="ExternalInput").ap()
    bo_d = nc.dram_tensor("bo", [U], F32, kind="ExternalInput").ap()

    out_d = nc.dram_tensor("out_o", [BPC, U], F32, kind="ExternalOutput").ap()
    h_d = nc.dram_tensor("h_o", [BPC, U], F32, kind="ExternalOutput").ap()

    with tile.TileContext(nc) as tc:
        _emit(nc, tc, locals())
    nc.compile()
    return nc


def _bcast_rows(ap_1d, rows, cols):
    """DMA source AP replicating a 1-D [cols] dram tensor across `rows` partitions."""
    return bass.AP(ap_1d.tensor, 0, [[0, rows], [1, cols]])


def _emit(nc, tc, d):
    ctxp_d, inp_d, h0_d = d["ctxp_d"], d["inp_d"], d["h0_d"]
    ctxn_d, h016_d = d["ctxn_d"], d["h016_d"]
    ua_d, wi_d, kern_d = d["ua_d"], d["wi_d"], d["kern_d"]
    wa8_d, va8_d, wadc_d = d["wa8_d"], d["va8_d"], d["wadc_d"]
    rec_d, attk_d, wo_d, id_d = (
        d["rec_d"], d["attk_d"], d["wo_d"], d["id_d"],
    )
    bi_d, bg_d, bau_d, bawt_d, bav_d, bo_d = (
        d["bi_d"], d["bg_d"], d["bau_d"], d["bawt_d"], d["bav_d"], d["bo_d"],
    )
    out_d, h_d = d["out_d"], d["h_d"]

    from contextlib import ExitStack

    es = ExitStack()
    wp = es.enter_context(tc.tile_pool(name="weights", bufs=1))
    gp = es.enter_context(tc.tile_pool(name="group", bufs=2))
    bp = es.enter_context(tc.tile_pool(name="perbatch", bufs=2))
    thp = es.enter_context(tc.tile_pool(name="tanh", bufs=2))
    natp = es.enter_context(tc.tile_pool(name="nat", bufs=3))
    natnp = es.enter_context(tc.tile_pool(name="natn", bufs=3))
    # PSUM budget: 8 banks = pS 2x2 + pSC 1x1 + pp 2x1 + pCv 1x1
    pS = es.enter_context(tc.tile_pool(name="psS", bufs=2, space="PSUM"))
    pSC = es.enter_context(tc.tile_pool(name="psSC", bufs=1, space="PSUM"))
    pp = es.enter_context(tc.tile_pool(name="psT", bufs=1, space="PSUM"))
    pCv = es.enter_context(tc.tile_pool(name="psCv", bufs=1, space="PSUM"))

    # ---- one-time loads (weights used in steady state) ----
    def load_kxm(pool, dram, rows, cols, tag, q=None):
        t = pool.tile([128, rows // 128, cols], F16, tag=tag, name=tag)
        src = bass.AP(dram.tensor, 0, [[cols, 128], [128 * cols, rows // 128], [1, cols]])
        (q or nc.gpsimd).dma_start(out=t, in_=src)
        return t

    id_sb = wp.tile([128, 128], F16)
    nc.scalar.dma_start(out=id_sb, in_=id_d)
    wa8_sb = wp.tile([128, KU, KU, 128], F8)
    nc.scalar.dma_start(out=wa8_sb, in_=wadc_d)
    va8_sb = wp.tile([128, 2, 2, 16], F8)
    nc.scalar.dma_start(out=va8_sb, in_=va8_d)

    def load_natT(pb_, q=None):
        t = natp.tile([128, KU, W], U16, tag="nat", name=f"natp{pb_}")
        src = bass.AP(ctxp_d.tensor, pb_ * W * U, [[U, W], [1, U]])
        (q or nc.sync).dma_start(out=t, in_=src, transpose=True)
        return t

    def load_natN(pb_):
        t = natnp.tile([128, 16, U], F8, tag="natn", name=f"natn{pb_}")
        src = bass.AP(ctxn_d.tensor, pb_ * T * U, [[U, 128], [128 * U, 16], [1, U]])
        nc.gpsimd.dma_start(out=t, in_=src)
        return t

    # batch 0's transposed context on the SP queue; batches 1-2 are issued
    # later on the scalar hwdge ring BEHIND the phase-0 weights, so their
    # 256B xbar packet storm cannot starve the startup-critical loads
    nat_pre = {pb_: load_natT(pb_) for pb_ in range(3)}
    natn_pre = {}

    bawt8 = wp.tile([128, KU, BPC], F32)
    nc.gpsimd.dma_start(out=bawt8, in_=bawt_d)
    bav_sb = wp.tile([1, 1], F32)
    nc.gpsimd.dma_start(out=bav_sb, in_=bav_d)

    # h_tm1 per group halves (partition slices >=4 are illegal on SBUF APs)
    h032g = []
    for g in range(2):
        t = wp.tile([4, U], F32, tag=f"h032g{g}", name=f"h032g{g}")
        nc.gpsimd.dma_start(out=t, in_=h0_d[g * 4:(g + 1) * 4, :])
        h032g.append(t)

    # ---- helpers ----
    def transpose_to(dst_f16, src, nrow, chunks):
        """PE-transpose src [nrow, chunks*128] f16 -> dst [128, chunks*nrow] f16."""
        pm = pS.tile([128, chunks * nrow], F16, tag="S", name="pm")
        for c in range(chunks):
            nc.tensor.transpose(
                pm[:, c * nrow:(c + 1) * nrow],
                src[0:nrow, c * 128:(c + 1) * 128],
                id_sb[0:nrow, 0:nrow],
            )
        nc.vector.tensor_copy(dst_f16, pm[:, 0:chunks * nrow])

    # ---- resident per-core intermediates ----
    qb = wp.tile([128, KU, BPC], F32)           # tanh bias (q + ba_w)^T
    xgg = [wp.tile([4, 3 * U], F32, tag=f"xg{g}", name=f"xg{g}") for g in range(2)]
    xgrzg = [wp.tile([4, 2 * U], F32, tag=f"xz{g}", name=f"xz{g}") for g in range(2)]

    # ---- phase 0 (scoped SBUF, reclaimed afterwards) ----
    with tc.tile_pool(name="phase0", bufs=1) as p0:
        inp16 = p0.tile([BPC, EMB], F16)
        nc.scalar.dma_start(out=inp16, in_=inp_d)
        h016 = p0.tile([BPC, U], F16)
        nc.scalar.dma_start(out=h016, in_=h016_d)
        bi8 = p0.tile([BPC, U], F32)
        nc.gpsimd.dma_start(out=bi8, in_=_bcast_rows(bi_d, BPC, U))
        bg4 = p0.tile([4, 3 * U], F32)
        nc.gpsimd.dma_start(out=bg4, in_=_bcast_rows(bg_d, 4, 3 * U))
        bau8 = p0.tile([BPC, U], F32)
        nc.gpsimd.dma_start(out=bau8, in_=_bcast_rows(bau_d, BPC, U))
        wi_sb = load_kxm(p0, wi_d, EMB, U, "wiw", q=nc.scalar)
        ua_sb = load_kxm(p0, ua_d, U, U, "uaw", q=nc.scalar)
        kern_sb = load_kxm(p0, kern_d, U, 3 * U, "kernw", q=nc.scalar)
        rec_sb = load_kxm(wp, rec_d, U, 3 * U, "recw", q=nc.scalar)

        inT = p0.tile([128, 2 * BPC], F16)      # layout [c, row]
        transpose_to(inT, inp16, BPC, 2)
        hT = p0.tile([128, KU * BPC], F16)
        transpose_to(hT, h016, BPC, KU)

        # x = inputs @ Wi + bi
        px = pS.tile([BPC, U], F32, tag="S", name="px")
        for c in range(2):
            nc.tensor.matmul(px, inT[:, c * BPC:(c + 1) * BPC], wi_sb[:, c, :],
                             start=(c == 0), stop=(c == 1))
        x16 = p0.tile([BPC, U], F16)
        nc.vector.tensor_add(x16, px, bi8)
        xT = p0.tile([128, KU * BPC], F16)
        transpose_to(xT, x16, BPC, KU)

        # xg / rec_zr computed per 4-batch half (legal partition starts)
        for g in range(2):
            for n in range(3):
                pg = pS.tile([4, U], F32, tag="S", name="pg")
                for c in range(KU):
                    nc.tensor.matmul(pg, xT[:, c * BPC + 4 * g:c * BPC + 4 * g + 4],
                                     kern_sb[:, c, n * U:(n + 1) * U],
                                     start=(c == 0), stop=(c == KU - 1))
                nc.vector.tensor_add(xgg[g][:, n * U:(n + 1) * U], pg,
                                     bg4[:, n * U:(n + 1) * U])
            for n in range(2):
                pr = pS.tile([4, U], F32, tag="S", name="pr")
                for c in range(KU):
                    nc.tensor.matmul(pr, hT[:, c * BPC + 4 * g:c * BPC + 4 * g + 4],
                                     rec_sb[:, c, n * U:(n + 1) * U],
                                     start=(c == 0), stop=(c == KU - 1))
                nc.vector.tensor_add(xgrzg[g][:, n * U:(n + 1) * U], pr,
                                     xgg[g][:, n * U:(n + 1) * U])

        # q = h @ Ua + ba_u ; transposed, +ba_w -> tanh bias [128, KU, BPC]
        pq = pS.tile([BPC, U], F32, tag="S", name="pq")
        for c in range(KU):
            nc.tensor.matmul(pq, hT[:, c * BPC:(c + 1) * BPC], ua_sb[:, c, :],
                             start=(c == 0), stop=(c == KU - 1))
        q16 = p0.tile([BPC, U], F16)
        nc.vector.tensor_add(q16, pq, bau8)
        pmq = pS.tile([128, KU * BPC], F16, tag="S", name="pmq")
        for c in range(KU):
            nc.tensor.transpose(pmq[:, c * BPC:(c + 1) * BPC],
                                q16[0:BPC, c * 128:(c + 1) * 128],
                                id_sb[0:BPC, 0:BPC])
        for c in range(KU):
            nc.vector.tensor_add(qb[:, c, :], pmq[:, c * BPC:(c + 1) * BPC],
                                 bawt8[:, c, :])

    for pb_ in range(3):
        natn_pre[pb_] = load_natN(pb_)
    attk_sb = load_kxm(wp, attk_d, U, 3 * U, "attkw")
    wo_sb = load_kxm(wp, wo_d, U, U, "wow")
    bo4 = wp.tile([4, U], F32)
    nc.gpsimd.dma_start(out=bo4, in_=_bcast_rows(bo_d, 4, U))

    def emit_group_post(grp, cvT16, h032, xg, gpp=None):

        # ---- group post: cg, gates, h, out ----

        gpool, gtag = (pp, "u") if gpp is None else gpp

        def mm_group(lhsT4, rhs_w, ncol_off):
            ptile = gpool.tile([4, U], F32, tag=gtag, name="ptile")
            for c in range(KU):
                nc.tensor.matmul(ptile, lhsT4[:, c, :],
                                 rhs_w[:, c, ncol_off:ncol_off + U],
                                 start=(c == 0), stop=(c == KU - 1))
            return ptile

        def sigmoid4(dst, pre):
            t1 = gp.tile([4, U], F32, tag="sig_t")
            nc.scalar.activation(t1, pre, AF.Tanh, scale=0.5)
            nc.vector.tensor_scalar(dst, t1, 0.5, 0.5, OP.mult, OP.add)

        xgrz = xgrzg[grp]
        # z and r gates: both cg matmuls first (independent PE work)
        pcg_z = mm_group(cvT16, attk_sb, 0)
        pcg_r = mm_group(cvT16, attk_sb, U)
        zpre = gp.tile([4, U], F32, tag="zpre")
        nc.vector.scalar_tensor_tensor(zpre, pcg_z, 1.0, xgrz[:, 0:U],
                                       OP.mult, OP.add)
        zg = gp.tile([4, U], F32, tag="zg")
        sigmoid4(zg, zpre)
        rpre = gp.tile([4, U], F32, tag="rpre")
        nc.vector.scalar_tensor_tensor(rpre, pcg_r, 1.0, xgrz[:, U:2 * U],
                                       OP.mult, OP.add)
        rg = gp.tile([4, U], F32, tag="rg")
        sigmoid4(rg, rpre)

        # rec_h = (r*h) @ Rh
        rh16 = gp.tile([4, U], F16, tag="rh16")
        nc.vector.tensor_mul(rh16, rg, h032)
        rhT = gp.tile([128, KU, 4], F16, tag="rhT")
        pmr = gpool.tile([128, KU * 4], F16, tag=gtag, name="pmr")
        for c in range(KU):
            nc.tensor.transpose(pmr[:, c * 4:(c + 1) * 4],
                                rh16[0:4, c * 128:(c + 1) * 128], id_sb[0:4, 0:4])
        nc.vector.tensor_copy(rhT, pmr[:, 0:KU * 4])
        prh = mm_group(rhT, rec_sb, 2 * U)

        # h_bar
        hpre = gp.tile([4, U], F32, tag="hpre")
        nc.vector.scalar_tensor_tensor(hpre, prh, 1.0, xg[:, 2 * U:3 * U],
                                       OP.mult, OP.add)
        pcg_h = mm_group(cvT16, attk_sb, 2 * U)
        nc.vector.tensor_add(hpre, hpre, pcg_h)
        hbar = gp.tile([4, U], F32, tag="hbar")
        nc.scalar.activation(hbar, hpre, AF.Tanh)

        # h = hbar + z*(h_tm1 - hbar)
        dd = gp.tile([4, U], F32, tag="dd")
        nc.vector.tensor_sub(dd, h032, hbar)
        h_out = gp.tile([4, U], F32, tag="h_out")
        nc.vector.scalar_tensor_tensor(h_out, dd, 1.0, zg, OP.mult, OP.mult)
        nc.vector.tensor_add(h_out, h_out, hbar)
        nc.sync.dma_start(out=h_d[grp * 4:(grp + 1) * 4, :], in_=h_out)

        # out = h @ Wo + bo
        h16 = gp.tile([4, U], F16, tag="h16")
        nc.vector.tensor_copy(h16, h_out)
        hT4 = gp.tile([128, KU, 4], F16, tag="hT4")
        pmh = gpool.tile([128, KU * 4], F16, tag=gtag, name="pmh")
        for c in range(KU):
            nc.tensor.transpose(pmh[:, c * 4:(c + 1) * 4],
                                h16[0:4, c * 128:(c + 1) * 128], id_sb[0:4, 0:4])
        nc.vector.tensor_copy(hT4, pmh[:, 0:KU * 4])
        pout = mm_group(hT4, wo_sb, 0)
        o_out = gp.tile([4, U], F32, tag="o_out")
        nc.vector.tensor_add(o_out, pout, bo4)
        nc.sync.dma_start(out=out_d[grp * 4:(grp + 1) * 4, :], in_=o_out)

    # ---- streaming over th-slots (software pipeline) ----
    # slot s = (b, th) with b = s//2, th = s%2.  Emit order per slot:
    #   Va+exp for slot s-1, scores+tanh for slot s, attn transposes for
    #   slot s-2, and cv finalize for the batch completed at slot s-2.
    # Every PE instruction then only depends on results >= 1 slot old, so
    # the in-order PE queue never blocks on ACT work of the same slot.
    NSLOT = 2 * BPC
    natT8_b = {}
    natN8_b = {}
    th8_s = {}
    expTh_s = {}
    zp_b = {}
    attnT8_b = {}
    psCv_b = {}
    cvT16_g = {}
    pending = []

    def emit_scores_tanh(s):
        b, th = s // 2, s % 2
        if th == 0:
            natT16 = nat_pre.pop(b) if b in nat_pre else None
            natT8_b[b] = natT16.bitcast(F8)
            natN8_b[b] = natn_pre.pop(b)
            if b + 3 < BPC:
                nat_pre[b + 3] = load_natT(b + 3)
            if b + 3 < BPC:
                natn_pre[b + 3] = load_natN(b + 3)
            zp_b[b] = bp.tile([1, 2], F32, tag="zpb", name="zp")
        natT8 = natT8_b[b]
        base = th * 1024

        def score_mms(ps_tiles, ms):
            for mi, m in enumerate(ms):
                for c in range(KU):
                    for half in range(2):
                        nc.tensor.matmul(
                            ps_tiles[mi][:, half * 512:(half + 1) * 512],
                            wa8_sb[:, c, m, :],
                            natT8[:, c,
                                  base + half * 512:base + (half + 1) * 512],
                            start=(c == 0), stop=(c == KU - 1),
                            perf_mode=mybir.MatmulPerfMode.DoubleColumn,
                        )

        th8 = thp.tile([128, KU, 1024], F8, tag="th")
        th8_s[s] = th8
        ps01 = [pS.tile([128, 1024], F32, tag="S", name=f"ps{mm}")
                for mm in range(2)]
        score_mms(ps01, [0, 1])
        for mi, m in enumerate([0, 1]):
            nc.scalar.activation(th8[:, m, :], ps01[mi], AF.Tanh,
                                 scale=1.0 / 16.0, bias=qb[:, m, b:b + 1])
        ps23 = [pS.tile([128, 1024], F32, tag="S", name=f"ps{mm + 2}")
                for mm in range(2)]
        score_mms(ps23, [2, 3])
        for mi, m in enumerate([2, 3]):
            nc.scalar.activation(th8[:, m, :], ps23[mi], AF.Tanh,
                                 scale=1.0 / 16.0, bias=qb[:, m, b:b + 1])

    def emit_va_exp(s):
        b, th = s // 2, s % 2
        th8 = th8_s.pop(s)
        # Va dot, fp8 DoubleRow: psc[t] = sum_u 16*Va[u] * th8[u, t]
        psc = pSC.tile([2, 1024], F32, tag="sc")
        for c in range(2):
            for half in range(2):
                nc.tensor.matmul(
                    psc[0:2, half * 512:(half + 1) * 512],
                    va8_sb[:, c, :, 0:2],
                    th8[:, 2 * c:2 * c + 2, half * 512:(half + 1) * 512],
                    start=(c == 0), stop=(c == 1),
                    perf_mode=mybir.MatmulPerfMode.DoubleRow,
                )
        expTh = bp.tile([1, 1024], F16, tag="expTh")
        expTh_s[s] = expTh
        nc.scalar.activation(expTh, psc[0:1, :], AF.Exp, scale=1.0 / 16.0,
                             bias=bav_sb[0:1, 0:1],
                             accum_out=zp_b[b][0:1, th:th + 1])

    def emit_attn_tp(s):
        b, th = s // 2, s % 2
        if th == 0:
            t = bp.tile([128, 16, 16], F8, tag="attnT8", name="attnT8")
            nc.vector.memset(t[:, :, 1:2], 0.0)
            attnT8_b[b] = t
        attnT8 = attnT8_b[b]
        expTh = expTh_s.pop(s)
        # transpose attn row -> [128, 8] columns, cast to fp8
        # (stride-2 pad: psum f16 writes need 4-byte alignment)
        pmA = pp.tile([128, 8, 2], F16, tag="u", name="pmA")
        for j in range(8):
            nc.tensor.transpose(pmA[:, j, 0:1],
                                expTh[0:1, j * 128:(j + 1) * 128],
                                id_sb[0:1, 0:1])
        nc.vector.tensor_copy(attnT8[:, 8 * th:8 * th + 8, 0:1]
                              .rearrange("p j one -> p (j one)"),
                              pmA[:, :, 0:1].rearrange("p j one -> p (j one)"))

    def emit_cv_fin(b):
        gi, grp = b % 4, b // 4
        if gi == 0:
            cvT16_g[grp] = gp.tile([128, KU, 4], F16, tag="cvT16", name="cvT16")
        cvT16 = cvT16_g[grp]
        attnT8 = attnT8_b.pop(b)
        natN8 = natN8_b.pop(b)
        del natT8_b[b]
        # cv = sum_t attn[t] * ctx[t, :] via fp8 DR matmuls over natural ctx
        psCv = pCv.tile([2, U], F32, tag="cv")
        for jp in range(8):
            nc.tensor.matmul(
                psCv,
                attnT8[:, 2 * jp:2 * jp + 2, 0:2],
                natN8[:, 2 * jp:2 * jp + 2, :],
                start=(jp == 0), stop=(jp == 7),
                perf_mode=mybir.MatmulPerfMode.DoubleRow,
            )
        # 1/Z and normalized transposed cv
        zp = zp_b.pop(b)
        zrec = bp.tile([1, 1], F32, tag="zrec")
        nc.vector.tensor_add(zrec, zp[:, 0:1], zp[:, 1:2])
        nc.vector.reciprocal(zrec, zrec)
        cv16 = bp.tile([1, U], F16, tag="cv16")
        nc.vector.tensor_scalar(cv16, psCv[0:1, :], zrec[0:1, 0:1], None, OP.mult)
        pmCv = pp.tile([128, KU, 2], F16, tag="u", name="pmCv")
        for c in range(KU):
            nc.tensor.transpose(pmCv[:, c, 0:1],
                                cv16[0:1, c * 128:(c + 1) * 128],
                                id_sb[0:1, 0:1])
        nc.vector.tensor_copy(cvT16[:, :, gi:gi + 1]
                              .rearrange("p c one -> p (c one)"),
                              pmCv[:, :, 0:1].rearrange("p c one -> p (c one)"))
        if gi == 3:
            pending.append((grp, cvT16))
        if len(pending) and gi == 1 and b > 1:
            g0, cv0 = pending.pop(0)
            emit_group_post(g0, cv0, h032g[g0], xgg[g0])

    for s in range(NSLOT + 2):
        if 1 <= s <= NSLOT:
            emit_va_exp(s - 1)
        if s < NSLOT:
            emit_scores_tanh(s)
        if s >= 2:
            s2 = s - 2
            if s2 < NSLOT:
                emit_attn_tp(s2)
            if s2 % 2 == 1:
                emit_cv_fin(s2 // 2)

    while pending:
        g0, cv0 = pending.pop(0)
        emit_group_post(g0, cv0, h032g[g0], xgg[g0], gpp=(pS, "S"))

    es.close()


_PROGRAM = None


def _get_program():
    global _PROGRAM
    if _PROGRAM is None:
        _PROGRAM = _build_program()
    return _PROGRAM


def make_in_maps(inputs, h_tm1, context, Wi, bi, kernel, recurrent_kernel,
                 attention_kernel, bias, Wa, ba_w, Ua, ba_u, Va, ba_v, Wo, bo):
    f32 = lambda x: np.ascontiguousarray(np.asarray(x, dtype=np.float32))
    f16 = lambda x: np.ascontiguousarray(np.asarray(x, dtype=np.float32).astype(np.float16))

    inputs = f32(inputs)
    h_tm1 = f32(h_tm1)

    # pack fp8 pairs along t into u16: packed[b, w, u] =
    #   fp8(ctx[b, 2w, u]) | fp8(ctx[b, 2w+1, u]) << 8
    f8np = mybir.dt.np(F8)
    ctx8 = np.asarray(context, np.float32).astype(f8np)               # [B,T,U]
    c8 = ctx8.view(np.uint8)
    ctxp = (c8[:, 0::2, :].astype(np.uint16)
            | (c8[:, 1::2, :].astype(np.uint16) << 8))                # [B,W,U]
    ctxp = np.ascontiguousarray(ctxp)

    wa32 = np.asarray(Wa, np.float32) * 16.0
    wa8dr = np.zeros((128, 2, 2, KU, 128), np.float32)
    for c in range(2):
        for i in range(2):
            for mc in range(KU):
                # lhsT[p, i, m] = Wa'[c*256 + i*128 + p, mc*128 + m]
                wa8dr[:, c, i, mc, :] = wa32[c * 256 + i * 128: c * 256 + (i + 1) * 128,
                                             mc * 128:(mc + 1) * 128]
    shared = {
        "wa8dr": np.ascontiguousarray(wa8dr.astype(f8np)),
        "wa8dc": np.ascontiguousarray(
            wa32.reshape(KU, 128, KU, 128).transpose(1, 0, 2, 3).astype(f8np)),
        "va8dr": np.ascontiguousarray(np.concatenate([
            (np.asarray(Va, np.float32).reshape(2, 2, 128) * 16.0)
            .transpose(2, 0, 1).reshape(128, 2, 2, 1),
            np.zeros((128, 2, 2, 15), np.float32)], axis=3).astype(f8np)),
        "ua16": f16(Ua), "wi16": f16(Wi),
        "kern16": f16(kernel), "rec16": f16(recurrent_kernel),
        "attk16": f16(attention_kernel), "wo16": f16(Wo),
        "ident16": np.eye(128, dtype=np.float16),
        "bi": f32(bi), "biasg": f32(bias), "ba_u": f32(ba_u),
        "ba_wt8": np.ascontiguousarray(np.repeat(
            np.asarray(ba_w, np.float32).reshape(KU, 128).T[:, :, None], BPC, axis=2)),
        "ba_v1": f32(ba_v).reshape(1, 1),
        "bo": f32(bo),
    }
    in_maps = []
    for i in range(NCORES):
        s = slice(i * BPC, (i + 1) * BPC)
        in_maps.append({
            "ctxp": ctxp[s], "ctx8n": ctx8[s],
            "inp16d": inputs[s].astype(np.float16),
            "h0": h_tm1[s], "h016d": h_tm1[s].astype(np.float16), **shared,
        })
    return in_maps


def kernel(**inputs):
    from concourse.bass_utils import run_bass_kernel_spmd

    nc = _get_program()
    in_maps = make_in_maps(**inputs)
    res = run_bass_kernel_spmd(nc, in_maps, list(range(NCORES)))
    out = np.concatenate([r["out_o"] for r in res.results], axis=0)
    h = np.concatenate([r["h_o"] for r in res.results], axis=0)
    return out.astype(np.float32), h.astype(np.float32)


if __name__ == "__main__":
    prog = _get_program()
    print("program built OK")


# revision 34
# speedup vs baseline: 1.2403x; 1.2403x over previous
"""Trainium2 Bass kernel for an attention-GRU cell (Bahdanau attention + GRU update).

Computation (per batch row b):
    x   = inputs @ Wi + bi
    xg  = x @ kernel + bias                       (split into x_z, x_r, x_h)
    q   = h_tm1 @ Ua + ba_u
    S   = tanh(context @ Wa + ba_w + q)           [t, U]
    sc  = S @ Va + ba_v                           [t]
    attn = softmax(sc)                            (scores bounded by ||Va||_1 -> no max-sub)
    cv  = sum_t attn * context                    [U]
    cg  = cv @ attention_kernel                   (c_z, c_r, c_h)
    z   = sigmoid(x_z + h@Rz + c_z) ; r = sigmoid(x_r + h@Rr + c_r)
    hb  = tanh(x_h + (r*h)@Rh + c_h)
    h   = z*h_tm1 + (1-z)*hb ; out = h @ Wo + bo

Sharding: batch (64) split across 8 cores, 8 batches/core, weights replicated.
Each core is fully independent (no collectives).

Key layout trick (v2): the host packs fp8e4(ctx[2w, u]) | fp8e4(ctx[2w+1, u])<<8
into a uint16 tensor [W=1024, U=512] per batch. The device runs the DMA XBAR
transpose (16x128 u16 tiles) per 128-u chunk, landing natT16 [128, KU, 1024]
u16 in SBUF whose fp8 bitcast is exactly ctx^T: natT8[p, c, t] = fp8 ctx[t,
128c+p] in natural t order. This removes all PE transposes and all ACT
PSUM->SBUF unpack copies of the baseline, and cuts ctx HBM traffic to 8.4MB
per core (fp8 instead of f32).

Per-batch pipeline:
  - scores: Wa (fp8, x16 host-scaled) stationary, natT8 moving, DoubleRow
    -> S^T chunks [128, 1024] f32 in PSUM; tanh on ACT with scale=1/16 and
    per-partition bias (q + ba_w)^T -> th16 [128, 4, 1024] f16
  - Va dot via matmul (lhsT = Va chunk [128,1]) -> scores [1, t] in PSUM
  - exp on ACT with fused accum_out -> softmax normalizer (no max-subtract:
    |score| <= ||Va||_1 ~ 8)
  - attn replicated across partitions (gpsimd partition_broadcast); ctx_vec
    on DVE via scalar_tensor_tensor with accum_out over natT8
  - gate math per 4-batch group on partitions 0..3; group-post emission is
    delayed behind the next batch's work to avoid PE head-of-line blocking
"""

import sys

if "/opt/trn_rl_repo" not in sys.path:
    sys.path.insert(0, "/opt/trn_rl_repo")

import numpy as np

import concourse.bass as bass
import concourse.mybir as mybir
import concourse.tile as tile
from concourse import bacc

F32 = mybir.dt.float32
F16 = mybir.dt.float16
U16 = mybir.dt.uint16
F8 = mybir.dt.float8e4
AF = mybir.ActivationFunctionType
OP = mybir.AluOpType

B = 64          # total batch
T = 2048        # context length
W = T // 2      # packed u16 rows
U = 512         # units
EMB = 256
NCORES = 8
BPC = B // NCORES   # batches per core
KU = U // 128       # 4 k-chunks over units


def _build_program():
    nc = bacc.Bacc("TRN2", target_bir_lowering=False, debug=False, num_devices=NCORES)

    # ---- DRAM I/O ----
    ctxp_d = nc.dram_tensor("ctxp", [BPC, W, U], U16, kind="ExternalInput").ap()
    ctxn_d = nc.dram_tensor("ctx8n", [BPC, T, U], F8, kind="ExternalInput").ap()
    inp_d = nc.dram_tensor("inp16d", [BPC, EMB], F16, kind="ExternalInput").ap()
    h0_d = nc.dram_tensor("h0", [BPC, U], F32, kind="ExternalInput").ap()
    h016_d = nc.dram_tensor("h016d", [BPC, U], F16, kind="ExternalInput").ap()

    wa8_d = nc.dram_tensor("wa8dr", [128, 2, 2, KU, 128], F8,
                           kind="ExternalInput").ap()
    va8_d = nc.dram_tensor("va8dr", [128, 2, 2, 16], F8,
                           kind="ExternalInput").ap()
    ua_d = nc.dram_tensor("ua16", [U, U], F16, kind="ExternalInput").ap()
    wi_d = nc.dram_tensor("wi16", [EMB, U], F16, kind="ExternalInput").ap()
    kern_d = nc.dram_tensor("kern16", [U, 3 * U], F16, kind="ExternalInput").ap()
    rec_d = nc.dram_tensor("rec16", [U, 3 * U], F16, kind="ExternalInput").ap()
    attk_d = nc.dram_tensor("attk16", [U, 3 * U], F16, kind="ExternalInput").ap()
    wo_d = nc.dram_tensor("wo16", [U, U], F16, kind="ExternalInput").ap()
    id_d = nc.dram_tensor("ident16", [128, 128], F16, kind="ExternalInput").ap()

    bi_d = nc.dram_tensor("bi", [U], F32, kind="ExternalInput").ap()
    bg_d = nc.dram_tensor("biasg", [3 * U], F32, kind="ExternalInput").ap()
    bau_d = nc.dram_tensor("ba_u", [U], F32, kind="ExternalInput").ap()
    bawt_d = nc.dram_tensor("ba_wt8", [128, KU, BPC], F32, kind="ExternalInput").ap()
    bav_d = nc.dram_tensor("ba_v1", [1, 1], F32, kind="ExternalInput").ap()
    bo_d = nc.dram_tensor("bo", [U], F32, kind="ExternalInput").ap()

    out_d = nc.dram_tensor("out_o", [BPC, U], F32, kind="ExternalOutput").ap()
    h_d = nc.dram_tensor("h_o", [BPC, U], F32, kind="ExternalOutput").ap()

    with tile.TileContext(nc) as tc:
        _emit(nc, tc, locals())
    nc.compile()
    return nc


def _bcast_rows(ap_1d, rows, cols):
    """DMA source AP replicating a 1-D [cols] dram tensor across `rows` partitions."""
    return bass.AP(ap_1d.tensor, 0, [[0, rows], [1, cols]])


def _emit(nc, tc, d):
    ctxp_d, inp_d, h0_d = d["ctxp_d"], d["inp_d"], d["h0_d"]
    ctxn_d, h016_d = d["ctxn_d"], d["h016_d"]
    ua_d, wi_d, kern_d = d["ua_d"], d["wi_d"], d["kern_d"]
    wa8_d, va8_d = d["wa8_d"], d["va8_d"]
    rec_d, attk_d, wo_d, id_d = (
        d["rec_d"], d["attk_d"], d["wo_d"], d["id_d"],
    )
    bi_d, bg_d, bau_d, bawt_d, bav_d, bo_d = (
        d["bi_d"], d["bg_d"], d["bau_d"], d["bawt_d"], d["bav_d"], d["bo_d"],
    )
    out_d, h_d = d["out_d"], d["h_d"]

    from contextlib import ExitStack

    es = ExitStack()
    wp = es.enter_context(tc.tile_pool(name="weights", bufs=1))
    gp = es.enter_context(tc.tile_pool(name="group", bufs=2))
    bp = es.enter_context(tc.tile_pool(name="perbatch", bufs=2))
    thp = es.enter_context(tc.tile_pool(name="tanh", bufs=2))
    natp = es.enter_context(tc.tile_pool(name="nat", bufs=3))
    natnp = es.enter_context(tc.tile_pool(name="natn", bufs=3))
    # PSUM budget: 8 banks = pS 2x2 + pSC 1x1 + pp 2x1 + pCv 1x1
    pS = es.enter_context(tc.tile_pool(name="psS", bufs=2, space="PSUM"))
    pSC = es.enter_context(tc.tile_pool(name="psSC", bufs=1, space="PSUM"))
    pp = es.enter_context(tc.tile_pool(name="psT", bufs=1, space="PSUM"))
    pCv = es.enter_context(tc.tile_pool(name="psCv", bufs=1, space="PSUM"))

    # ---- one-time loads (weights used in steady state) ----
    def load_kxm(pool, dram, rows, cols, tag, q=None):
        t = pool.tile([128, rows // 128, cols], F16, tag=tag, name=tag)
        src = bass.AP(dram.tensor, 0, [[cols, 128], [128 * cols, rows // 128], [1, cols]])
        (q or nc.gpsimd).dma_start(out=t, in_=src)
        return t

    id_sb = wp.tile([128, 128], F16)
    nc.scalar.dma_start(out=id_sb, in_=id_d)
    wa8_sb = wp.tile([128, 2, 2, KU, 128], F8)
    nc.scalar.dma_start(out=wa8_sb, in_=wa8_d)
    va8_sb = wp.tile([128, 2, 2, 16], F8)
    nc.scalar.dma_start(out=va8_sb, in_=va8_d)

    def load_natT(pb_, q=None):
        t = natp.tile([128, KU, W], U16, tag="nat", name=f"natp{pb_}")
        src = bass.AP(ctxp_d.tensor, pb_ * W * U, [[U, W], [1, U]])
        (q or nc.sync).dma_start(out=t, in_=src, transpose=True)
        return t

    def load_natN(pb_):
        t = natnp.tile([128, 16, U], F8, tag="natn", name=f"natn{pb_}")
        src = bass.AP(ctxn_d.tensor, pb_ * T * U, [[U, 128], [128 * U, 16], [1, U]])
        nc.gpsimd.dma_start(out=t, in_=src)
        return t

    # batch 0's transposed context on the SP queue; batches 1-2 are issued
    # later on the scalar hwdge ring BEHIND the phase-0 weights, so their
    # 256B xbar packet storm cannot starve the startup-critical loads
    nat_pre = {pb_: load_natT(pb_) for pb_ in range(3)}
    natn_pre = {}

    bawt8 = wp.tile([128, KU, BPC], F32)
    nc.gpsimd.dma_start(out=bawt8, in_=bawt_d)
    bav_sb = wp.tile([1, 1], F32)
    nc.gpsimd.dma_start(out=bav_sb, in_=bav_d)

    # h_tm1 per group halves (partition slices >=4 are illegal on SBUF APs)
    h032g = []
    for g in range(2):
        t = wp.tile([4, U], F32, tag=f"h032g{g}", name=f"h032g{g}")
        nc.gpsimd.dma_start(out=t, in_=h0_d[g * 4:(g + 1) * 4, :])
        h032g.append(t)

    # ---- helpers ----
    def transpose_to(dst_f16, src, nrow, chunks):
        """PE-transpose src [nrow, chunks*128] f16 -> dst [128, chunks*nrow] f16."""
        pm = pS.tile([128, chunks * nrow], F16, tag="S", name="pm")
        for c in range(chunks):
            nc.tensor.transpose(
                pm[:, c * nrow:(c + 1) * nrow],
                src[0:nrow, c * 128:(c + 1) * 128],
                id_sb[0:nrow, 0:nrow],
            )
        nc.vector.tensor_copy(dst_f16, pm[:, 0:chunks * nrow])

    # ---- resident per-core intermediates ----
    qb = wp.tile([128, KU, BPC], F32)           # tanh bias (q + ba_w)^T
    xgg = [wp.tile([4, 3 * U], F32, tag=f"xg{g}", name=f"xg{g}") for g in range(2)]
    xgrzg = [wp.tile([4, 2 * U], F32, tag=f"xz{g}", name=f"xz{g}") for g in range(2)]

    # ---- phase 0 (scoped SBUF, reclaimed afterwards) ----
    with tc.tile_pool(name="phase0", bufs=1) as p0:
        inp16 = p0.tile([BPC, EMB], F16)
        nc.scalar.dma_start(out=inp16, in_=inp_d)
        h016 = p0.tile([BPC, U], F16)
        nc.scalar.dma_start(out=h016, in_=h016_d)
        bi8 = p0.tile([BPC, U], F32)
        nc.gpsimd.dma_start(out=bi8, in_=_bcast_rows(bi_d, BPC, U))
        bg4 = p0.tile([4, 3 * U], F32)
        nc.gpsimd.dma_start(out=bg4, in_=_bcast_rows(bg_d, 4, 3 * U))
        bau8 = p0.tile([BPC, U], F32)
        nc.gpsimd.dma_start(out=bau8, in_=_bcast_rows(bau_d, BPC, U))
        wi_sb = load_kxm(p0, wi_d, EMB, U, "wiw", q=nc.scalar)
        ua_sb = load_kxm(p0, ua_d, U, U, "uaw", q=nc.scalar)
        kern_sb = load_kxm(p0, kern_d, U, 3 * U, "kernw", q=nc.scalar)
        rec_sb = load_kxm(wp, rec_d, U, 3 * U, "recw", q=nc.scalar)

        inT = p0.tile([128, 2 * BPC], F16)      # layout [c, row]
        transpose_to(inT, inp16, BPC, 2)
        hT = p0.tile([128, KU * BPC], F16)
        transpose_to(hT, h016, BPC, KU)

        # x = inputs @ Wi + bi
        px = pS.tile([BPC, U], F32, tag="S", name="px")
        for c in range(2):
            nc.tensor.matmul(px, inT[:, c * BPC:(c + 1) * BPC], wi_sb[:, c, :],
                             start=(c == 0), stop=(c == 1))
        x16 = p0.tile([BPC, U], F16)
        nc.vector.tensor_add(x16, px, bi8)
        xT = p0.tile([128, KU * BPC], F16)
        transpose_to(xT, x16, BPC, KU)

        # xg / rec_zr computed per 4-batch half (legal partition starts)
        for g in range(2):
            for n in range(3):
                pg = pS.tile([4, U], F32, tag="S", name="pg")
                for c in range(KU):
                    nc.tensor.matmul(pg, xT[:, c * BPC + 4 * g:c * BPC + 4 * g + 4],
                                     kern_sb[:, c, n * U:(n + 1) * U],
                                     start=(c == 0), stop=(c == KU - 1))
                nc.vector.tensor_add(xgg[g][:, n * U:(n + 1) * U], pg,
                                     bg4[:, n * U:(n + 1) * U])
            for n in range(2):
                pr = pS.tile([4, U], F32, tag="S", name="pr")
                for c in range(KU):
                    nc.tensor.matmul(pr, hT[:, c * BPC + 4 * g:c * BPC + 4 * g + 4],
                                     rec_sb[:, c, n * U:(n + 1) * U],
                                     start=(c == 0), stop=(c == KU - 1))
                nc.vector.tensor_add(xgrzg[g][:, n * U:(n + 1) * U], pr,
                                     xgg[g][:, n * U:(n + 1) * U])

        # q = h @ Ua + ba_u ; transposed, +ba_w -> tanh bias [128, KU, BPC]
        pq = pS.tile([BPC, U], F32, tag="S", name="pq")
        for c in range(KU):
            nc.tensor.matmul(pq, hT[:, c * BPC:(c + 1) * BPC], ua_sb[:, c, :],
                             start=(c == 0), stop=(c == KU - 1))
        q16 = p0.tile([BPC, U], F16)
        nc.vector.tensor_add(q16, pq, bau8)
        pmq = pS.tile([128, KU * BPC], F16, tag="S", name="pmq")
        for c in range(KU):
            nc.tensor.transpose(pmq[:, c * BPC:(c + 1) * BPC],
                                q16[0:BPC, c * 128:(c + 1) * 128],
                                id_sb[0:BPC, 0:BPC])
        for c in range(KU):
            nc.vector.tensor_add(qb[:, c, :], pmq[:, c * BPC:(c + 1) * BPC],
                                 bawt8[:, c, :])

    for pb_ in range(3):
        natn_pre[pb_] = load_natN(pb_)
    attk_sb = load_kxm(wp, attk_d, U, 3 * U, "attkw")
    wo_sb = load_kxm(wp, wo_d, U, U, "wow")
    bo4 = wp.tile([4, U], F32)
    nc.gpsimd.dma_start(out=bo4, in_=_bcast_rows(bo_d, 4, U))

    def emit_group_post(grp, cvT16, h032, xg, gpp=None):

        # ---- group post: cg, gates, h, out ----

        gpool, gtag = (pp, "u") if gpp is None else gpp

        def mm_group(lhsT4, rhs_w, ncol_off):
            ptile = gpool.tile([4, U], F32, tag=gtag, name="ptile")
            for c in range(KU):
                nc.tensor.matmul(ptile, lhsT4[:, c, :],
                                 rhs_w[:, c, ncol_off:ncol_off + U],
                                 start=(c == 0), stop=(c == KU - 1))
            return ptile

        def sigmoid4(dst, pre):
            t1 = gp.tile([4, U], F32, tag="sig_t")
            nc.scalar.activation(t1, pre, AF.Tanh, scale=0.5)
            nc.vector.tensor_scalar(dst, t1, 0.5, 0.5, OP.mult, OP.add)

        xgrz = xgrzg[grp]
        # z and r gates: both cg matmuls first (independent PE work)
        pcg_z = mm_group(cvT16, attk_sb, 0)
        pcg_r = mm_group(cvT16, attk_sb, U)
        zpre = gp.tile([4, U], F32, tag="zpre")
        nc.vector.scalar_tensor_tensor(zpre, pcg_z, 1.0, xgrz[:, 0:U],
                                       OP.mult, OP.add)
        zg = gp.tile([4, U], F32, tag="zg")
        sigmoid4(zg, zpre)
        rpre = gp.tile([4, U], F32, tag="rpre")
        nc.vector.scalar_tensor_tensor(rpre, pcg_r, 1.0, xgrz[:, U:2 * U],
                                       OP.mult, OP.add)
        rg = gp.tile([4, U], F32, tag="rg")
        sigmoid4(rg, rpre)

        # rec_h = (r*h) @ Rh
        rh16 = gp.tile([4, U], F16, tag="rh16")
        nc.vector.tensor_mul(rh16, rg, h032)
        rhT = gp.tile([128, KU, 4], F16, tag="rhT")
        pmr = gpool.tile([128, KU * 4], F16, tag=gtag, name="pmr")
        for c in range(KU):
            nc.tensor.transpose(pmr[:, c * 4:(c + 1) * 4],
                                rh16[0:4, c * 128:(c + 1) * 128], id_sb[0:4, 0:4])
        nc.vector.tensor_copy(rhT, pmr[:, 0:KU * 4])
        prh = mm_group(rhT, rec_sb, 2 * U)

        # h_bar
        hpre = gp.tile([4, U], F32, tag="hpre")
        nc.vector.scalar_tensor_tensor(hpre, prh, 1.0, xg[:, 2 * U:3 * U],
                                       OP.mult, OP.add)
        pcg_h = mm_group(cvT16, attk_sb, 2 * U)
        nc.vector.tensor_add(hpre, hpre, pcg_h)
        hbar = gp.tile([4, U], F32, tag="hbar")
        nc.scalar.activation(hbar, hpre, AF.Tanh)

        # h = hbar + z*(h_tm1 - hbar)
        dd = gp.tile([4, U], F32, tag="dd")
        nc.vector.tensor_sub(dd, h032, hbar)
        h_out = gp.tile([4, U], F32, tag="h_out")
        nc.vector.scalar_tensor_tensor(h_out, dd, 1.0, zg, OP.mult, OP.mult)
        nc.vector.tensor_add(h_out, h_out, hbar)
        nc.sync.dma_start(out=h_d[grp * 4:(grp + 1) * 4, :], in_=h_out)

        # out = h @ Wo + bo
        h16 = gp.tile([4, U], F16, tag="h16")
        nc.vector.tensor_copy(h16, h_out)
        hT4 = gp.tile([128, KU, 4], F16, tag="hT4")
        pmh = gpool.tile([128, KU * 4], F16, tag=gtag, name="pmh")
        for c in range(KU):
            nc.tensor.transpose(pmh[:, c * 4:(c + 1) * 4],
                                h16[0:4, c * 128:(c + 1) * 128], id_sb[0:4, 0:4])
        nc.vector.tensor_copy(hT4, pmh[:, 0:KU * 4])
        pout = mm_group(hT4, wo_sb, 0)
        o_out = gp.tile([4, U], F32, tag="o_out")
        nc.vector.tensor_add(o_out, pout, bo4)
        nc.sync.dma_start(out=out_d[grp * 4:(grp + 1) * 4, :], in_=o_out)

    # ---- streaming over th-slots (software pipeline) ----
    # slot s = (b, th) with b = s//2, th = s%2.  Emit order per slot:
    #   Va+exp for slot s-1, scores+tanh for slot s, attn transposes for
    #   slot s-2, and cv finalize for the batch completed at slot s-2.
    # Every PE instruction then only depends on results >= 1 slot old, so
    # the in-order PE queue never blocks on ACT work of the same slot.
    NSLOT = 2 * BPC
    natT8_b = {}
    natN8_b = {}
    th8_s = {}
    expTh_s = {}
    zp_b = {}
    attnT8_b = {}
    psCv_b = {}
    cvT16_g = {}
    pending = []

    def emit_scores_tanh(s):
        b, th = s // 2, s % 2
        if th == 0:
            natT16 = nat_pre.pop(b) if b in nat_pre else None
            natT8_b[b] = natT16.bitcast(F8)
            natN8_b[b] = natn_pre.pop(b)
            if b + 3 < BPC:
                nat_pre[b + 3] = load_natT(b + 3)
            if b + 3 < BPC:
                natn_pre[b + 3] = load_natN(b + 3)
            zp_b[b] = bp.tile([1, 2], F32, tag="zpb", name="zp")
        natT8 = natT8_b[b]
        base = th * 1024

        def score_mms(ps_tiles, ms):
            for mi, m in enumerate(ms):
                for c in range(2):
                    for half in range(2):
                        nc.tensor.matmul(
                            ps_tiles[mi][:, half * 512:(half + 1) * 512],
                            wa8_sb[:, c, :, m, :],
                            natT8[:, 2 * c:2 * c + 2,
                                  base + half * 512:base + (half + 1) * 512],
                            start=(c == 0), stop=(c == 1),
                            perf_mode=mybir.MatmulPerfMode.DoubleRow,
                        )

        th8 = thp.tile([128, KU, 1024], F8, tag="th")
        th8_s[s] = th8
        ps01 = [pS.tile([128, 1024], F32, tag="S", name=f"ps{mm}")
                for mm in range(2)]
        score_mms(ps01, [0, 1])
        for mi, m in enumerate([0, 1]):
            nc.scalar.activation(th8[:, m, :], ps01[mi], AF.Tanh,
                                 scale=1.0 / 16.0, bias=qb[:, m, b:b + 1])
        ps23 = [pS.tile([128, 1024], F32, tag="S", name=f"ps{mm + 2}")
                for mm in range(2)]
        score_mms(ps23, [2, 3])
        for mi, m in enumerate([2, 3]):
            nc.scalar.activation(th8[:, m, :], ps23[mi], AF.Tanh,
                                 scale=1.0 / 16.0, bias=qb[:, m, b:b + 1])

    def emit_va_exp(s):
        b, th = s // 2, s % 2
        th8 = th8_s.pop(s)
        # Va dot, fp8 DoubleRow: psc[t] = sum_u 16*Va[u] * th8[u, t]
        psc = pSC.tile([2, 1024], F32, tag="sc")
        for c in range(2):
            for half in range(2):
                nc.tensor.matmul(
                    psc[0:2, half * 512:(half + 1) * 512],
                    va8_sb[:, c, :, 0:2],
                    th8[:, 2 * c:2 * c + 2, half * 512:(half + 1) * 512],
                    start=(c == 0), stop=(c == 1),
                    perf_mode=mybir.MatmulPerfMode.DoubleRow,
                )
        expTh = bp.tile([1, 1024], F16, tag="expTh")
        expTh_s[s] = expTh
        nc.scalar.activation(expTh, psc[0:1, :], AF.Exp, scale=1.0 / 16.0,
                             bias=bav_sb[0:1, 0:1],
                             accum_out=zp_b[b][0:1, th:th + 1])

    def emit_attn_tp(s):
        b, th = s // 2, s % 2
        if th == 0:
            t = bp.tile([128, 16, 16], F8, tag="attnT8", name="attnT8")
            nc.vector.memset(t[:, :, 1:2], 0.0)
            attnT8_b[b] = t
        attnT8 = attnT8_b[b]
        expTh = expTh_s.pop(s)
        # transpose attn row -> [128, 8] columns, cast to fp8
        # (stride-2 pad: psum f16 writes need 4-byte alignment)
        pmA = pp.tile([128, 8, 2], F16, tag="u", name="pmA")
        for j in range(8):
            nc.tensor.transpose(pmA[:, j, 0:1],
                                expTh[0:1, j * 128:(j + 1) * 128],
                                id_sb[0:1, 0:1])
        nc.vector.tensor_copy(attnT8[:, 8 * th:8 * th + 8, 0:1]
                              .rearrange("p j one -> p (j one)"),
                              pmA[:, :, 0:1].rearrange("p j one -> p (j one)"))

    def emit_cv_fin(b):
        gi, grp = b % 4, b // 4
        if gi == 0:
            cvT16_g[grp] = gp.tile([128, KU, 4], F16, tag="cvT16", name="cvT16")
        cvT16 = cvT16_g[grp]
        attnT8 = attnT8_b.pop(b)
        natN8 = natN8_b.pop(b)
        del natT8_b[b]
        # cv = sum_t attn[t] * ctx[t, :] via fp8 DR matmuls over natural ctx
        psCv = pCv.tile([2, U], F32, tag="cv")
        for jp in range(8):
            nc.tensor.matmul(
                psCv,
                attnT8[:, 2 * jp:2 * jp + 2, 0:2],
                natN8[:, 2 * jp:2 * jp + 2, :],
                start=(jp == 0), stop=(jp == 7),
                perf_mode=mybir.MatmulPerfMode.DoubleRow,
            )
        # 1/Z and normalized transposed cv
        zp = zp_b.pop(b)
        zrec = bp.tile([1, 1], F32, tag="zrec")
        nc.vector.tensor_add(zrec, zp[:, 0:1], zp[:, 1:2])
        nc.vector.reciprocal(zrec, zrec)
        cv16 = bp.tile([1, U], F16, tag="cv16")
        nc.vector.tensor_scalar(cv16, psCv[0:1, :], zrec[0:1, 0:1], None, OP.mult)
        pmCv = pp.tile([128, KU, 2], F16, tag="u", name="pmCv")
        for c in range(KU):
            nc.tensor.transpose(pmCv[:, c, 0:1],
                                cv16[0:1, c * 128:(c + 1) * 128],
                                id_sb[0:1, 0:1])
        nc.vector.tensor_copy(cvT16[:, :, gi:gi + 1]
                              .rearrange("p c one -> p (c one)"),
                              pmCv[:, :, 0:1].rearrange("p c one -> p (c one)"))
        if gi == 3:
            pending.append((grp, cvT16))
        if len(pending) and gi == 1 and b > 1:
            g0, cv0 = pending.pop(0)
            emit_group_post(g0, cv0, h032g[g0], xgg[g0])

    for s in range(NSLOT + 2):
        if 1 <= s <= NSLOT:
            emit_va_exp(s - 1)
        if s < NSLOT:
            emit_scores_tanh(s)
        if s >= 2:
            s2 = s - 2
            if s2 < NSLOT:
                emit_attn_tp(s2)
            if s2 % 2 == 1:
                emit_cv_fin(s2 // 2)

    while pending:
        g0, cv0 = pending.pop(0)
        emit_group_post(g0, cv0, h032g[g0], xgg[g0], gpp=(pS, "S"))

    es.close()


_PROGRAM = None


def _get_program():
    global _PROGRAM
    if _PROGRAM is None:
        _PROGRAM = _build_program()
    return _PROGRAM


def make_in_maps(inputs, h_tm1, context, Wi, bi, kernel, recurrent_kernel,
                 attention_kernel, bias, Wa, ba_w, Ua, ba_u, Va, ba_v, Wo, bo):
    f32 = lambda x: np.ascontiguousarray(np.asarray(x, dtype=np.float32))
    f16 = lambda x: np.ascontiguousarray(np.asarray(x, dtype=np.float32).astype(np.float16))

    inputs = f32(inputs)
    h_tm1 = f32(h_tm1)

    # pack fp8 pairs along t into u16: packed[b, w, u] =
    #   fp8(ctx[b, 2w, u]) | fp8(ctx[b, 2w+1, u]) << 8
    f8np = mybir.dt.np(F8)
    ctx8 = np.asarray(context, np.float32).astype(f8np)               # [B,T,U]
    c8 = ctx8.view(np.uint8)
    ctxp = (c8[:, 0::2, :].astype(np.uint16)
            | (c8[:, 1::2, :].astype(np.uint16) << 8))                # [B,W,U]
    ctxp = np.ascontiguousarray(ctxp)

    wa32 = np.asarray(Wa, np.float32) * 16.0
    wa8dr = np.zeros((128, 2, 2, KU, 128), np.float32)
    for c in range(2):
        for i in range(2):
            for mc in range(KU):
                # lhsT[p, i, m] = Wa'[c*256 + i*128 + p, mc*128 + m]
                wa8dr[:, c, i, mc, :] = wa32[c * 256 + i * 128: c * 256 + (i + 1) * 128,
                                             mc * 128:(mc + 1) * 128]
    shared = {
        "wa8dr": np.ascontiguousarray(wa8dr.astype(f8np)),
        "va8dr": np.ascontiguousarray(np.concatenate([
            (np.asarray(Va, np.float32).reshape(2, 2, 128) * 16.0)
            .transpose(2, 0, 1).reshape(128, 2, 2, 1),
            np.zeros((128, 2, 2, 15), np.float32)], axis=3).astype(f8np)),
        "ua16": f16(Ua), "wi16": f16(Wi),
        "kern16": f16(kernel), "rec16": f16(recurrent_kernel),
        "attk16": f16(attention_kernel), "wo16": f16(Wo),
        "ident16": np.eye(128, dtype=np.float16),
        "bi": f32(bi), "biasg": f32(bias), "ba_u": f32(ba_u),
        "ba_wt8": np.ascontiguousarray(np.repeat(
            np.asarray(ba_w, np.float32).reshape(KU, 128).T[:, :, None], BPC, axis=2)),
        "ba_v1": f32(ba_v).reshape(1, 1),
        "bo": f32(bo),
    }
    in_maps = []
    for i in range(NCORES):
        s = slice(i * BPC, (i + 1) * BPC)
        in_maps.append({
            "ctxp": ctxp[s], "ctx8n": ctx8[s],
            "inp16d": inputs[s].astype(np.float16),
            "h0": h_tm1[s], "h016d": h_tm1[s].astype(np.float16), **shared,
        })
    return in_maps


def kernel(**inputs):
    from concourse.bass_utils import run_bass_kernel_spmd

    nc = _get_program()
    in_maps = make_in_maps(**inputs)
    res = run_bass_kernel_spmd(nc, in_maps, list(range(NCORES)))
    out = np.concatenate([r["out_o"] for r in res.results], axis=0)
    h = np.concatenate([r["h_o"] for r in res.results], axis=0)
    return out.astype(np.float32), h.astype(np.float32)


if __name__ == "__main__":
    prog = _get_program()
    print("program built OK")
